# revision 19
# baseline (speedup 1.0000x reference)
"""CGCNNConv (gnn_message_passing) on 8 TRN2 NeuronCores.

Sharding: edges sorted by dst and partitioned into 8 contiguous dst-ranges
(one per core). Node features and weights replicated. Each core:
  - builds projection tables h_src = node @ W_src, h_dst = node @ W_dst in DRAM
  - pass 1 over its edges: indirect-DMA gathers h_src[src], h_dst[dst],
    edge projection via matmul, m accumulated in PSUM (identity-matmul adds),
    BN1 stats via ones/Gram matmuls on PE, m stored transposed to DRAM
  - BN1 stats AllReduce (tiny), per-feature affine folded into ACT scale/bias
  - pass 2: sigmoid x softplus (ACT LUTs, group-batched table switches),
    scatter-add to the core's own node range via one-hot matmuls into PSUM
  - BN2 stats AllReduce, final softplus(node + bn(h)) for own nodes
Host reassembles the 8 node-slices.
"""
import sys

for _p in ("/opt/trn_rl_repo",):
    if _p not in sys.path:
        sys.path.insert(0, _p)

import numpy as np
import concourse.bass as bass
import concourse.mybir as mybir
from concourse.tile import TileContext
from concourse.masks import make_identity
from concourse.bass_utils import run_bass_kernel_spmd

P = 128
ST = 8            # 128-edge tiles per supertile (pass 1 unit)
GH = 4            # supertiles per pass-2 group (table-switch batching)
FP = mybir.dt.float32
I32 = mybir.dt.int32
EPS = 1e-5

# Full-size problem constants (grading size); overridable for sim tests.
N_NODES = 50000
N_EDGES = 800000
NF = 64
EF = 32
NCORES = 8


def _ceil(a, b):
    return -(-a // b)


def host_prep(node_feats, edge_feats, src, dst, n_nodes, n_cores):
    """Sort edges by dst, partition by dst-range, build per-core device arrays
    and the uniform (cross-core) scatter schedule."""
    E = src.shape[0]
    n_own = n_nodes // n_cores
    order = np.argsort(dst, kind="stable")
    src_s = src[order].astype(np.int64)
    dst_s = dst[order].astype(np.int64)
    ef_s = edge_feats[order]

    bounds = np.searchsorted(dst_s, np.arange(n_cores + 1) * n_own)
    e_counts = np.diff(bounds)
    e_max = int(e_counts.max())
    e_pad = max(_ceil(e_max, ST * P) * (ST * P), ST * P)
    T = e_pad // P
    nch = _ceil(n_own, P)

    cores = []
    # per-core per-tile median chunk (for uniform primary map)
    med = np.full((n_cores, T), -1, np.int64)
    # per-core per-chunk tile ranges
    t0s = np.zeros((n_cores, nch), np.int64)
    t1s = np.zeros((n_cores, nch), np.int64)
    for k in range(n_cores):
        lo, hi = int(bounds[k]), int(bounds[k + 1])
        ek = hi - lo
        sg = np.full(e_pad, n_nodes, np.int64)
        dg = np.full(e_pad, n_nodes, np.int64)
        dl = np.full(e_pad, -1e9, np.float64)
        efk = np.zeros((e_pad, EF), np.float32)
        sg[:ek] = src_s[lo:hi]
        dg[:ek] = dst_s[lo:hi]
        dloc = dst_s[lo:hi] - k * n_own
        dl[:ek] = dloc
        efk[:ek] = ef_s[lo:hi]
        cores.append(dict(ek=ek, sg=sg, dg=dg, dl=dl, efk=efk))

        ntile_real = _ceil(ek, P)
        for t in range(ntile_real):
            mid = min(t * P + P // 2, ek - 1)
            med[k, t] = dloc[mid] // P
        cl = np.searchsorted(dloc, np.arange(nch) * P)
        ch = np.searchsorted(dloc, np.minimum((np.arange(nch) + 1) * P, n_own))
        t0s[k] = cl // P
        t1s[k] = np.maximum(_ceil(1, 1) * ((ch + P - 1) // P), t0s[k] + 1)

    # uniform primary map: median over cores of per-tile chunk (valid entries)
    primary = np.zeros(T, np.int64)
    for t in range(T):
        v = med[:, t][med[:, t] >= 0]
        primary[t] = int(np.median(v)) if len(v) else nch - 1
    # uniform chunk tile ranges (union)
    t0u = t0s.min(axis=0)
    t1u = t1s.max(axis=0)
    # enforce monotone non-decreasing ends so close-order is ascending
    for c in range(1, nch):
        t1u[c] = max(t1u[c], t1u[c - 1])

    # per-tile scatter schedule: list of (chunk, start, stop)
    sched = [[] for _ in range(T)]
    for c in range(nch):
        for t in range(int(t0u[c]), int(t1u[c])):
            sched[t].append((c, t == int(t0u[c]), t == int(t1u[c]) - 1))
    maxopen = 0
    openc = 0
    for t in range(T):
        for (c, st, sp) in sched[t]:
            if st:
                openc += 1
        maxopen = max(maxopen, openc)
        for (c, st, sp) in sched[t]:
            if sp:
                openc -= 1

    # device arrays per core
    inmaps = []
    for k in range(n_cores):
        d = cores[k]
        srcT = d["sg"].reshape(T, P).T.astype(np.int32).copy()
        dstTi = d["dg"].reshape(T, P).T.astype(np.int32).copy()
        dstT = d["dl"].reshape(T, P).T.astype(np.float32).copy()
        shift = (d["dl"].reshape(T, P) - (primary * P)[:, None]).T
        dstShiftT = np.clip(shift, -1e9, 1e9).astype(np.float32).copy()
        edgeT = np.ascontiguousarray(d["efk"].T)
        inmaps.append(dict(srcT=srcT, dstTi=dstTi, dstT=dstT,
                           dstShiftT=dstShiftT, edgeT=edgeT))

    meta = dict(e_pad=e_pad, T=T, nch=nch, n_own=n_own,
                primary=primary, sched=sched, maxopen=maxopen,
                e_total=E)
    return inmaps, meta


def _legalize_waits(nc, cap=1):
    """This container's walrus codegen rejects instructions carrying more
    than one semaphore wait ("Too many sync wait commands"). Hoist extra
    waits onto dedicated same-engine NoOps in front of the instruction —
    sync semantics are identical (sequencer processes waits in order)."""
    E = mybir.EngineType
    capped = {E.PE, E.Pool, E.DVE, E.Activation, E.SP}
    for f in nc.m.functions:
        for blk in f.blocks:
            out = []
            for inst in list(blk.instructions):
                si = inst.sync_info
                if (si is not None and inst.engine in capped
                        and len(si.on_wait) > cap):
                    waits = list(si.on_wait)
                    for k, w in enumerate(waits[:-cap]):
                        out.append(mybir.InstNoOp(
                            name=f"{inst.name}-ws{k}", engine=inst.engine,
                            bass_nofuse=True,
                            sync_info=mybir.SyncInfo(on_wait=[w], on_update=[])))
                    inst.sync_info = mybir.SyncInfo(
                        on_wait=waits[-cap:], on_update=list(si.on_update))
                out.append(inst)
            blk.instructions = out


def build_program(meta, n_nodes, n_cores):
    e_pad = meta["e_pad"]
    T = meta["T"]
    nch = meta["nch"]
    n_own = meta["n_own"]
    primary = meta["primary"]
    sched = meta["sched"]
    e_total = meta["e_total"]
    NST = T // ST
    F2 = 2 * NF                      # 128
    nt_rows = _ceil(n_nodes + P, P) * P   # table rows incl. zero pad row
    n_node_tiles = _ceil(n_nodes, P)
    inv_e = 1.0 / float(e_total)
    inv_n = 1.0 / float(n_nodes)

    nc = bass.Bass(num_devices=n_cores)

    # ---- external I/O ----
    nodeT_d = nc.dram_tensor("nodeT", [NF, n_nodes], FP, kind="ExternalInput")
    node_own_d = nc.dram_tensor("node_own", [n_own, NF], FP, kind="ExternalInput")
    wcat_d = nc.dram_tensor("W_cat", [NF, 2 * F2], FP, kind="ExternalInput")
    wedge_d = nc.dram_tensor("W_edge", [EF, F2], FP, kind="ExternalInput")
    gbm_d = nc.dram_tensor("gb_m", [F2, 2], FP, kind="ExternalInput")
    gbn_d = nc.dram_tensor("gb_n", [NF, 2], FP, kind="ExternalInput")
    srcT_d = nc.dram_tensor("srcT", [P, T], I32, kind="ExternalInput")
    dstTi_d = nc.dram_tensor("dstTi", [P, T], I32, kind="ExternalInput")
    dstT_d = nc.dram_tensor("dstT", [P, T], FP, kind="ExternalInput")
    dstShiftT_d = nc.dram_tensor("dstShiftT", [P, T], FP, kind="ExternalInput")
    edgeT_d = nc.dram_tensor("edgeT", [EF, e_pad], FP, kind="ExternalInput")
    out_d = nc.dram_tensor("out", [n_own, NF], FP, kind="ExternalOutput")

    # ---- internal DRAM ----
    hsrc_d = nc.dram_tensor("hsrc_tab", [nt_rows, F2], FP)
    hdst_d = nc.dram_tensor("hdst_tab", [nt_rows, F2], FP)
    mT_d = nc.dram_tensor("mT", [F2, e_pad], FP)
    st1_in = nc.dram_tensor("st1_in", [F2, 2], FP)
    st1_out = nc.dram_tensor("st1_out", [F2, 2], FP)
    st2_in = nc.dram_tensor("st2_in", [NF, 2], FP)
    st2_out = nc.dram_tensor("st2_out", [NF, 2], FP)
    groups = [list(range(n_cores))]

    with TileContext(nc) as tc:
        with tc.tile_pool(name="const", bufs=1) as cs, \
             tc.tile_pool(name="stats_ps", bufs=1, space="PSUM") as stps:
            # ---------- constants ----------
            ident = cs.tile([P, P], FP)
            make_identity(nc, ident[:])
            ones_col = cs.tile([P, 1], FP)
            nc.gpsimd.memset(ones_col[:], 1.0)
            ones_row = cs.tile([1, P], FP)
            nc.gpsimd.memset(ones_row[:], 1.0)
            iota_i = cs.tile([P, 1], I32)
            nc.gpsimd.iota(iota_i[:], pattern=[[0, 1]], base=0, channel_multiplier=1)
            iota_f = cs.tile([P, 1], FP)
            nc.vector.tensor_copy(iota_f[:], iota_i[:])
            wcat_sb = cs.tile([NF, 2 * F2], FP)
            nc.sync.dma_start(out=wcat_sb[:], in_=wcat_d[:])
            wedge_sb = cs.tile([EF, F2], FP)
            nc.sync.dma_start(out=wedge_sb[:], in_=wedge_d[:])
            gbm_sb = cs.tile([F2, 2], FP)
            nc.sync.dma_start(out=gbm_sb[:], in_=gbm_d[:])
            gbn_sb = cs.tile([NF, 2], FP)
            nc.sync.dma_start(out=gbn_sb[:], in_=gbn_d[:])
            srcT_sb = cs.tile([P, T], I32)
            nc.sync.dma_start(out=srcT_sb[:], in_=srcT_d[:])
            dstTi_sb = cs.tile([P, T], I32)
            nc.sync.dma_start(out=dstTi_sb[:], in_=dstTi_d[:])
            dstT_sb = cs.tile([P, T], FP)
            nc.sync.dma_start(out=dstT_sb[:], in_=dstT_d[:])
            dstShiftT_sb = cs.tile([P, T], FP)
            nc.sync.dma_start(out=dstShiftT_sb[:], in_=dstShiftT_d[:])
            h_sb = cs.tile([P, nch * NF], FP)       # scatter result, resident
            a1_sb = cs.tile([F2, 1], FP)            # BN1 scale (per-partition)
            c1_sb = cs.tile([F2, 1], FP)            # BN1 bias
            A2b = cs.tile([P, NF], FP)              # BN2 scale row-bcast
            C2b = cs.tile([P, NF], FP)

            # iota_row[i, j] = j  (PE transpose of broadcast column)
            with tc.tile_pool(name="init_ps", bufs=1, space="PSUM") as ips:
                it_ps = ips.tile([P, P], FP, space="PSUM")
                nc.tensor.transpose(out=it_ps[:], in_=iota_f[:].to_broadcast([P, P]),
                                    identity=ident[:])
                iota_row = cs.tile([P, P], FP)
                nc.scalar.copy(out=iota_row[:], in_=it_ps[:])
            iotaRep = cs.tile([P, ST * P], FP)
            nc.vector.tensor_copy(
                out=iotaRep[:].rearrange("p (k f) -> p k f", k=ST),
                in_=iota_row[:].rearrange("p (o f) -> p o f", o=1).to_broadcast([P, ST, P]))

            # persistent PSUM stats accumulators packed into one 2KB bank:
            # cols 0:128 gram1, 128:129 sum1, 256:320 gram2 (64p), 320:321 sum2
            stats_tile = stps.tile([P, 512], FP, space="PSUM")
            gram_ps = stats_tile[:, 0:F2]
            sum_ps = stats_tile[:, F2:F2 + 1]
            gram2_ps = stats_tile[0:NF, 256:256 + NF]
            sum2_ps = stats_tile[0:NF, 256 + NF:256 + NF + 1]

            # ---------- phase A: projection tables ----------
            NB = 8
            with tc.tile_pool(name="pa_sb", bufs=3) as pa, \
                 tc.tile_pool(name="pa_ps", bufs=3, space="PSUM") as pap:
                zt = pa.tile([P, F2], FP, tag="zt")
                nc.gpsimd.memset(zt[:], 0.0)
                nc.sync.dma_start(out=hsrc_d[n_nodes:n_nodes + P, :], in_=zt[:])
                nc.sync.dma_start(out=hdst_d[n_nodes:n_nodes + P, :], in_=zt[:])
                for i0 in range(0, n_node_tiles, NB):
                    nb = min(NB, n_node_tiles - i0)
                    cols0 = i0 * P
                    ncols = min(nb * P, n_nodes - cols0)
                    nt_sb = pa.tile([NF, NB * P], FP, tag="nt")
                    nc.sync.dma_start(out=nt_sb[:, :ncols],
                                      in_=nodeT_d[:, cols0:cols0 + ncols])
                    stage_s = pa.tile([P, NB * P], FP, tag="ss")
                    stage_d = pa.tile([P, NB * P], FP, tag="sd")
                    for j in range(nb):
                        pp = pap.tile([P, 2 * F2], FP, space="PSUM", tag="pp")
                        nc.tensor.matmul(out=pp[:], lhsT=nt_sb[:, j * P:(j + 1) * P],
                                         rhs=wcat_sb[:], start=True, stop=True)
                        nc.scalar.copy(out=stage_s[:, j * F2:(j + 1) * F2],
                                       in_=pp[:, 0:F2])
                        nc.vector.tensor_copy(out=stage_d[:, j * F2:(j + 1) * F2],
                                              in_=pp[:, F2:2 * F2])
                    rows = ncols
                    if rows == nb * P:
                        nc.sync.dma_start(
                            out=hsrc_d[cols0:cols0 + rows, :].rearrange(
                                "(j p) f -> p j f", p=P),
                            in_=stage_s[:].rearrange("p (j f) -> p j f", f=F2))
                        nc.sync.dma_start(
                            out=hdst_d[cols0:cols0 + rows, :].rearrange(
                                "(j p) f -> p j f", p=P),
                            in_=stage_d[:].rearrange("p (j f) -> p j f", f=F2))
                    else:
                        for j in range(nb):
                            r = min(P, rows - j * P)
                            if r <= 0:
                                break
                            nc.sync.dma_start(
                                out=hsrc_d[cols0 + j * P:cols0 + j * P + r, :],
                                in_=stage_s[:r, j * F2:(j + 1) * F2])
                            nc.sync.dma_start(
                                out=hdst_d[cols0 + j * P:cols0 + j * P + r, :],
                                in_=stage_d[:r, j * F2:(j + 1) * F2])

            # ---------- phase B: pass 1 over edges ----------
            with tc.tile_pool(name="pb_sb", bufs=3) as pb, \
                 tc.tile_pool(name="pb_ps", bufs=2, space="PSUM") as pbp, \
                 tc.tile_pool(name="pb_ps2", bufs=1, space="PSUM") as pbp2:
                for s in range(NST):
                    c0 = s * ST * P
                    # NOTE: multi-index offset APs ([128,K]) are broken on real
                    # HW (descriptor/offset walk mismatch) — one indirect DMA
                    # per 128-edge tile with a [128,1] offset is the reliable
                    # form (same as concourse/kernels/tile_scatter_add.py).
                    gsrc = pb.tile([P, ST * F2], FP, tag="gsrc")
                    gdst = pb.tile([P, ST * F2], FP, tag="gdst")
                    for j in range(ST):
                        tt = s * ST + j
                        nc.gpsimd.indirect_dma_start(
                            out=gsrc[:, j * F2:(j + 1) * F2],
                            out_offset=None, in_=hsrc_d[:],
                            in_offset=bass.IndirectOffsetOnAxis(
                                ap=srcT_sb[:, tt:tt + 1], axis=0))
                        nc.gpsimd.indirect_dma_start(
                            out=gdst[:, j * F2:(j + 1) * F2],
                            out_offset=None, in_=hdst_d[:],
                            in_offset=bass.IndirectOffsetOnAxis(
                                ap=dstTi_sb[:, tt:tt + 1], axis=0))
                    et_sb = pb.tile([EF, ST * P], FP, tag="et")
                    nc.sync.dma_start(out=et_sb[:],
                                      in_=edgeT_d[:, c0:c0 + ST * P])
                    m_ps = pbp.tile([P, ST * F2], FP, space="PSUM", tag="mps")
                    # start=True zeroes the whole 2KB bank: only the first
                    # matmul per bank (j=0 and j=4) may set it.
                    for j in range(ST):
                        nc.tensor.matmul(out=m_ps[:, j * F2:(j + 1) * F2],
                                         lhsT=et_sb[:, j * P:(j + 1) * P],
                                         rhs=wedge_sb[:],
                                         start=(j % 4 == 0), stop=False,
                                         skip_group_check=True)
                    H = ST * F2 // 2
                    for half in range(2):
                        sl = slice(half * H, (half + 1) * H)
                        nc.tensor.matmul(out=m_ps[:, sl], lhsT=ident[:],
                                         rhs=gsrc[:, sl], start=False, stop=False,
                                         skip_group_check=True)
                        nc.tensor.matmul(out=m_ps[:, sl], lhsT=ident[:],
                                         rhs=gdst[:, sl], start=False, stop=True,
                                         skip_group_check=True)
                    m_sb = pb.tile([P, ST * F2], FP, tag="msb")
                    if s % 2 == 0:
                        nc.scalar.copy(out=m_sb[:], in_=m_ps[:])
                    else:
                        nc.vector.tensor_copy(out=m_sb[:], in_=m_ps[:])
                    # BN1 stats on PE
                    for j in range(ST):
                        mj = m_sb[:, j * F2:(j + 1) * F2]
                        nc.tensor.matmul(out=gram_ps[:], lhsT=mj, rhs=mj,
                                         start=(s == 0 and j == 0),
                                         stop=(s == NST - 1 and j == ST - 1),
                                         skip_group_check=True)
                        # sum shares the stats bank: gram's first start=True
                        # already zeroed it, so never start here.
                        nc.tensor.matmul(out=sum_ps[:], lhsT=mj, rhs=ones_col[:],
                                         start=False,
                                         stop=(s == NST - 1 and j == ST - 1),
                                         skip_group_check=True)
                    # transpose m -> [feat, edge] and store
                    mt_ps = pbp2.tile([P, ST * F2], FP, space="PSUM", tag="mtps")
                    for j in range(ST):
                        nc.tensor.matmul(out=mt_ps[:, j * P:(j + 1) * P],
                                         lhsT=m_sb[:, j * F2:(j + 1) * F2],
                                         rhs=ident[:], is_transpose=True,
                                         start=(j % 4 == 0), stop=(j % 4 == 3),
                                         skip_group_check=True)
                    mt_sb = pb.tile([P, ST * P], FP, tag="mtsb")
                    if s % 2 == 0:
                        nc.vector.tensor_copy(out=mt_sb[:], in_=mt_ps[:])
                    else:
                        nc.scalar.copy(out=mt_sb[:], in_=mt_ps[:])
                    nc.sync.dma_start(out=mT_d[:, c0:c0 + ST * P], in_=mt_sb[:])

            # ---------- phase C: BN1 finalize ----------
            with tc.tile_pool(name="pc_sb", bufs=1) as pc:
                G = pc.tile([F2, F2], FP)
                nc.scalar.copy(out=G[:], in_=gram_ps[:])
                nc.vector.tensor_tensor(out=G[:], in0=G[:], in1=ident[:],
                                        op=mybir.AluOpType.mult)
                S2 = pc.tile([F2, 1], FP)
                nc.vector.reduce_sum(out=S2[:], in_=G[:], axis=mybir.AxisListType.X)
                S1 = pc.tile([F2, 1], FP)
                nc.scalar.copy(out=S1[:], in_=sum_ps[:])
                nc.sync.dma_start(out=st1_in[:, 0:1], in_=S1[:])
                nc.sync.dma_start(out=st1_in[:, 1:2], in_=S2[:])
                nc.gpsimd.collective_compute(
                    "AllReduce", mybir.AluOpType.add, replica_groups=groups,
                    ins=[st1_in[:]], outs=[st1_out[:]])
                stv = pc.tile([F2, 2], FP)
                nc.sync.dma_start(out=stv[:], in_=st1_out[:])
                mean = pc.tile([F2, 1], FP)
                nc.vector.tensor_scalar_mul(out=mean[:], in0=stv[:, 0:1],
                                            scalar1=inv_e)
                var = pc.tile([F2, 1], FP)
                nc.vector.tensor_scalar_mul(out=var[:], in0=stv[:, 1:2],
                                            scalar1=inv_e)
                msq = pc.tile([F2, 1], FP)
                nc.vector.tensor_tensor(out=msq[:], in0=mean[:], in1=mean[:],
                                        op=mybir.AluOpType.mult)
                nc.vector.tensor_tensor(out=var[:], in0=var[:], in1=msq[:],
                                        op=mybir.AluOpType.subtract)
                nc.vector.tensor_scalar_add(out=var[:], in0=var[:], scalar1=EPS)
                # rsqrt = exp(-0.5 ln(var))
                nc.scalar.activation(out=var[:], in_=var[:],
                                     func=mybir.ActivationFunctionType.Ln)
                nc.vector.tensor_scalar_mul(out=var[:], in0=var[:], scalar1=-0.5)
                nc.scalar.activation(out=var[:], in_=var[:],
                                     func=mybir.ActivationFunctionType.Exp)
                nc.vector.tensor_tensor(out=a1_sb[:], in0=var[:], in1=gbm_sb[:, 0:1],
                                        op=mybir.AluOpType.mult)
                tmpc = pc.tile([F2, 1], FP)
                nc.vector.tensor_tensor(out=tmpc[:], in0=mean[:], in1=a1_sb[:],
                                        op=mybir.AluOpType.mult)
                nc.vector.tensor_tensor(out=c1_sb[:], in0=gbm_sb[:, 1:2], in1=tmpc[:],
                                        op=mybir.AluOpType.subtract)

            # ---------- phase D: pass 2 + scatter ----------
            CW = GH * ST * P            # columns (edges) per group
            NG = _ceil(e_pad, CW)
            psum_open = {}
            nclosed = [0]
            with tc.tile_pool(name="pd_sb", bufs=2) as pd, \
                 tc.tile_pool(name="pd_sb2", bufs=2) as pd2, \
                 tc.tile_pool(name="pd_ps", bufs=2, space="PSUM") as pdp, \
                 tc.tile_pool(name="pd_chps",
                              bufs=min(5, max(3, meta["maxopen"] + 1)),
                              space="PSUM") as chp:
                for g in range(NG):
                    gc0 = g * CW
                    gcols = min(CW, e_pad - gc0)
                    gtiles = gcols // P
                    mt2 = pd.tile([F2, CW], FP, tag="mt2")
                    nc.sync.dma_start(out=mt2[:, :gcols],
                                      in_=mT_d[:, gc0:gc0 + gcols])
                    sig = pd.tile([NF, CW], FP, tag="sig")
                    nc.scalar.activation(out=sig[:, :gcols], in_=mt2[0:NF, :gcols],
                                         func=mybir.ActivationFunctionType.Sigmoid,
                                         scale=a1_sb[0:NF, :], bias=c1_sb[0:NF, :])
                    # softplus(x) = ln(exp(x) + 1); exp/ln share one ACT table
                    # set (safe range: |x| stays small post-BN). Ln runs in
                    # place on es; the product lands in place on sig.
                    es = pd.tile([NF, CW], FP, tag="es")
                    nc.scalar.activation(out=es[:, :gcols], in_=mt2[NF:F2, :gcols],
                                         func=mybir.ActivationFunctionType.Exp,
                                         scale=a1_sb[NF:F2, :], bias=c1_sb[NF:F2, :])
                    nc.scalar.activation(out=es[:, :gcols], in_=es[:, :gcols],
                                         func=mybir.ActivationFunctionType.Ln,
                                         bias=1.0)
                    msg = sig
                    nc.vector.tensor_tensor(out=msg[:, :gcols], in0=sig[:, :gcols],
                                            in1=es[:, :gcols],
                                            op=mybir.AluOpType.mult)
                    for b in range(_ceil(gtiles, ST)):
                        bt0 = g * GH * ST + b * ST          # first global tile
                        btiles = min(ST, gtiles - b * ST)
                        # batched one-hot S for primary chunks
                        S_sb = pd2.tile([P, ST * P], FP, tag="S")
                        nc.vector.tensor_tensor(
                            out=S_sb[:, :btiles * P].rearrange(
                                "p (k f) -> p k f", f=P),
                            in0=iotaRep[:, :btiles * P].rearrange(
                                "p (k f) -> p k f", f=P),
                            in1=dstShiftT_sb[:, bt0:bt0 + btiles].rearrange(
                                "p (k o) -> p k o", o=1).to_broadcast(
                                    [P, btiles, P]),
                            op=mybir.AluOpType.is_equal)
                        # transpose msg tiles back to [edge, feat]
                        tr_ps = pdp.tile([P, ST * NF], FP, space="PSUM", tag="tr")
                        for j in range(btiles):
                            col = b * ST * P + j * P
                            nc.tensor.matmul(
                                out=tr_ps[:, j * NF:(j + 1) * NF],
                                lhsT=msg[:, col:col + P],
                                rhs=ident[:NF, :NF], is_transpose=True,
                                start=(j == 0), stop=(j == btiles - 1),
                                skip_group_check=True)
                        msg_sb = pd2.tile([P, ST * NF], FP, tag="msgsb")
                        if b % 2 == 0:
                            nc.scalar.copy(out=msg_sb[:, :btiles * NF],
                                           in_=tr_ps[:, :btiles * NF])
                        else:
                            nc.vector.tensor_copy(out=msg_sb[:, :btiles * NF],
                                                  in_=tr_ps[:, :btiles * NF])
                        for j in range(btiles):
                            t = bt0 + j
                            rhs = msg_sb[:, j * NF:(j + 1) * NF]
                            for (c, c_start, c_stop) in sched[t]:
                                if c == primary[t]:
                                    lhsT = S_sb[:, j * P:(j + 1) * P]
                                else:
                                    S_x = pd2.tile([P, P], FP, tag="Sx")
                                    nc.vector.tensor_scalar(
                                        out=S_x[:], in0=iota_row[:],
                                        scalar1=float(c * P),
                                        scalar2=dstT_sb[:, t:t + 1],
                                        op0=mybir.AluOpType.add,
                                        op1=mybir.AluOpType.is_equal)
                                    lhsT = S_x[:]
                                if c_start:
                                    psum_open[c] = chp.tile(
                                        [P, NF], FP, space="PSUM", tag="ch",
                                        name=f"chps_{c}")
                                nc.tensor.matmul(out=psum_open[c][:], lhsT=lhsT,
                                                 rhs=rhs, start=c_start,
                                                 stop=c_stop,
                                                 skip_group_check=True)
                                if c_stop:
                                    hc = h_sb[:, c * NF:(c + 1) * NF]
                                    if c % 2 == 0:
                                        nc.scalar.copy(out=hc, in_=psum_open[c][:])
                                    else:
                                        nc.vector.tensor_copy(out=hc,
                                                              in_=psum_open[c][:])
                                    # gram2/sum2 share the stats bank, zeroed
                                    # by gram1's first start in pass 1.
                                    nc.tensor.matmul(
                                        out=gram2_ps[:], lhsT=hc, rhs=hc,
                                        start=False,
                                        stop=(nclosed[0] == nch - 1),
                                        skip_group_check=True)
                                    nc.tensor.matmul(
                                        out=sum2_ps[:], lhsT=hc, rhs=ones_col[:],
                                        start=False,
                                        stop=(nclosed[0] == nch - 1),
                                        skip_group_check=True)
                                    nclosed[0] += 1
                                    del psum_open[c]

            # ---------- phase E: BN2 finalize ----------
            with tc.tile_pool(name="pe_sb", bufs=1) as pe, \
                 tc.tile_pool(name="pe_ps", bufs=2, space="PSUM") as pep:
                G2 = pe.tile([NF, NF], FP)
                nc.scalar.copy(out=G2[:], in_=gram2_ps[:])
                nc.vector.tensor_tensor(out=G2[:], in0=G2[:], in1=ident[:NF, :NF],
                                        op=mybir.AluOpType.mult)
                S2b = pe.tile([NF, 1], FP)
                nc.vector.reduce_sum(out=S2b[:], in_=G2[:], axis=mybir.AxisListType.X)
                S1b = pe.tile([NF, 1], FP)
                nc.scalar.copy(out=S1b[:], in_=sum2_ps[:])
                nc.sync.dma_start(out=st2_in[:, 0:1], in_=S1b[:])
                nc.sync.dma_start(out=st2_in[:, 1:2], in_=S2b[:])
                nc.gpsimd.collective_compute(
                    "AllReduce", mybir.AluOpType.add, replica_groups=groups,
                    ins=[st2_in[:]], outs=[st2_out[:]])
                stv2 = pe.tile([NF, 2], FP)
                nc.sync.dma_start(out=stv2[:], in_=st2_out[:])
                mean2 = pe.tile([NF, 1], FP)
                nc.vector.tensor_scalar_mul(out=mean2[:], in0=stv2[:, 0:1],
                                            scalar1=inv_n)
                var2 = pe.tile([NF, 1], FP)
                nc.vector.tensor_scalar_mul(out=var2[:], in0=stv2[:, 1:2],
                                            scalar1=inv_n)
                msq2 = pe.tile([NF, 1], FP)
                nc.vector.tensor_tensor(out=msq2[:], in0=mean2[:], in1=mean2[:],
                                        op=mybir.AluOpType.mult)
                nc.vector.tensor_tensor(out=var2[:], in0=var2[:], in1=msq2[:],
                                        op=mybir.AluOpType.subtract)
                nc.vector.tensor_scalar_add(out=var2[:], in0=var2[:], scalar1=EPS)
                nc.scalar.activation(out=var2[:], in_=var2[:],
                                     func=mybir.ActivationFunctionType.Ln)
                nc.vector.tensor_scalar_mul(out=var2[:], in0=var2[:], scalar1=-0.5)
                nc.scalar.activation(out=var2[:], in_=var2[:],
                                     func=mybir.ActivationFunctionType.Exp)
                a2 = pe.tile([NF, 1], FP)
                nc.vector.tensor_tensor(out=a2[:], in0=var2[:], in1=gbn_sb[:, 0:1],
                                        op=mybir.AluOpType.mult)
                c2 = pe.tile([NF, 1], FP)
                nc.vector.tensor_tensor(out=c2[:], in0=mean2[:], in1=a2[:],
                                        op=mybir.AluOpType.mult)
                nc.vector.tensor_tensor(out=c2[:], in0=gbn_sb[:, 1:2], in1=c2[:],
                                        op=mybir.AluOpType.subtract)
                # broadcast a2,c2 rows into [P, NF] tiles via PE outer products
                rp = pep.tile([1, NF], FP, space="PSUM", tag="rp")
                rs_ = pe.tile([1, NF], FP)
                nc.tensor.matmul(out=rp[:], lhsT=a2[:], rhs=ident[:NF, :NF],
                                 start=True, stop=True)
                nc.scalar.copy(out=rs_[:], in_=rp[:])
                bp = pep.tile([P, NF], FP, space="PSUM", tag="bp")
                nc.tensor.matmul(out=bp[:], lhsT=ones_row[:], rhs=rs_[:],
                                 start=True, stop=True)
                nc.scalar.copy(out=A2b[:], in_=bp[:])
                rp2 = pep.tile([1, NF], FP, space="PSUM", tag="rp")
                rs2_ = pe.tile([1, NF], FP)
                nc.tensor.matmul(out=rp2[:], lhsT=c2[:], rhs=ident[:NF, :NF],
                                 start=True, stop=True)
                nc.scalar.copy(out=rs2_[:], in_=rp2[:])
                bp2 = pep.tile([P, NF], FP, space="PSUM", tag="bp")
                nc.tensor.matmul(out=bp2[:], lhsT=ones_row[:], rhs=rs2_[:],
                                 start=True, stop=True)
                nc.scalar.copy(out=C2b[:], in_=bp2[:])

            # ---------- phase F: final output ----------
            with tc.tile_pool(name="pf_sb", bufs=3) as pf:
                for c in range(nch):
                    rows = min(P, n_own - c * P)
                    nf_sb = pf.tile([P, NF], FP, tag="nf")
                    nc.sync.dma_start(out=nf_sb[:rows, :],
                                      in_=node_own_d[c * P:c * P + rows, :])
                    t1 = pf.tile([P, NF], FP, tag="t1")
                    nc.vector.tensor_tensor(out=t1[:], in0=h_sb[:, c * NF:(c + 1) * NF],
                                            in1=A2b[:], op=mybir.AluOpType.mult)
                    nc.vector.tensor_tensor(out=t1[:], in0=t1[:], in1=C2b[:],
                                            op=mybir.AluOpType.add)
                    nc.vector.tensor_tensor(out=t1[:rows, :], in0=t1[:rows, :],
                                            in1=nf_sb[:rows, :],
                                            op=mybir.AluOpType.add)
                    o_sb = pf.tile([P, NF], FP, tag="o")
                    nc.scalar.activation(out=o_sb[:rows, :], in_=t1[:rows, :],
                                         func=mybir.ActivationFunctionType.Exp)
                    nc.scalar.activation(out=o_sb[:rows, :], in_=o_sb[:rows, :],
                                         func=mybir.ActivationFunctionType.Ln,
                                         bias=1.0)
                    nc.sync.dma_start(out=out_d[c * P:c * P + rows, :],
                                      in_=o_sb[:rows, :])
    _legalize_waits(nc)
    return nc


def kernel(node_feats, edge_feats, src, dst,
           W_src, b_src, W_dst, b_dst, W_edge, b_edge,
           gamma_m, beta_m, gamma_n, beta_n,
           n_cores=NCORES, _run=None):
    node_feats = np.asarray(node_feats, np.float32)
    edge_feats = np.asarray(edge_feats, np.float32)
    src = np.asarray(src)
    dst = np.asarray(dst)
    n_nodes = node_feats.shape[0]
    n_own = n_nodes // n_cores

    # biases b_src/b_dst/b_edge are no-ops: BatchNorm immediately subtracts
    # the per-feature mean, and variance is shift-invariant.
    inmaps, meta = host_prep(node_feats, edge_feats, src, dst, n_nodes, n_cores)

    nodeT = np.ascontiguousarray(node_feats.T)
    W_cat = np.concatenate([np.asarray(W_src, np.float32),
                            np.asarray(W_dst, np.float32)], axis=1)
    W_edge = np.asarray(W_edge, np.float32)
    gb_m = np.stack([np.asarray(gamma_m, np.float32),
                     np.asarray(beta_m, np.float32)], axis=1)
    gb_n = np.stack([np.asarray(gamma_n, np.float32),
                     np.asarray(beta_n, np.float32)], axis=1)
    for k in range(n_cores):
        inmaps[k].update(
            nodeT=nodeT, W_cat=W_cat, W_edge=W_edge, gb_m=gb_m, gb_n=gb_n,
            node_own=np.ascontiguousarray(
                node_feats[k * n_own:(k + 1) * n_own]))

    nc = build_program(meta, n_nodes, n_cores)
    if _run is not None:                       # sim hook for testing
        results = _run(nc, inmaps)
    else:
        results = run_bass_kernel_spmd(nc, inmaps,
                                       list(range(n_cores))).results
    out = np.concatenate([np.asarray(results[k]["out"])
                          for k in range(n_cores)], axis=0)
    return out.astype(np.float32)
